# revision 1
# baseline (speedup 1.0000x reference)
"""Causal self-attention on 8 axon-tunneled TRN2 NeuronCores.

Sharding: core c -> (batch b = c//2, head-group g = c%2); host sums the two
head-group partial yT outputs per batch. All matmuls float32r (full PE rate,
~1.5e-4 err). Transpose-free S^T attention layout; softmax denominator via a
ones-column in v; 1/s broadcast via K=1 PE matmul. Attention processes head
PAIRS with interleaved row-groups (0-1 / 2-3) so LDWEIGHTS of one head's
scores overlaps the other head's matmuls."""
import numpy as np

B, T, D = 4, 2048, 1024
NH_LOCAL = 8
HD = 64
CL = 512
P = 128
CC = D // P
TC = T // P
TT = T // 512
NPAIR = 4

_CACHE = {}


def _emit_qkv(nc, tc, mybir, r, aps, qT_sb, kT_sb, v_sb):
    f32 = mybir.dt.float32
    f32r = mybir.dt.float32r
    xT_r, wqT_r, wkT_r, wvT_r = aps
    with tc.tile_pool(name=f"p1x{r}", bufs=1) as p1x, \
         tc.tile_pool(name=f"p1q{r}", bufs=3, space="PSUM") as p1q, \
         tc.tile_pool(name=f"p1ps{r}", bufs=2, space="PSUM") as p1ps:
        xT_sb = p1x.tile([P, CC, T], f32r, tag="xT")

        # qT / kT first: PE can start as soon as the first weight slice lands
        with tc.tile_pool(name=f"p1w{r}", bufs=2) as p1w:
            w_slices = []
            for p_i in range(2):  # prefetch first two pairs' weights
                for w_r, wtag in ((wqT_r, "wq"), (wkT_r, "wk")):
                    w_sl = p1w.tile([P, CC, P], f32r, tag=f"{wtag}{p_i}")
                    nc.sync.dma_start(
                        w_sl[:], w_r[:, :, p_i * P:(p_i + 1) * P])
                    w_slices.append(w_sl)
            # xT on two queues for 2x DMA bandwidth; chunk 0 first on the
            # gpsimd queue so it lands in parallel with the weight slices
            for cc in range(CC):
                eng = nc.gpsimd if cc % 2 == 0 else nc.sync
                eng.dma_start(xT_sb[:, cc, :], xT_r[:, cc, :])
            for p_i in range(NPAIR):
                for wi, (w_r, dst, wtag) in enumerate(
                        ((wqT_r, qT_sb, "wq"), (wkT_r, kT_sb, "wk"))):
                    if p_i < 2:
                        w_sl = w_slices[p_i * 2 + wi]
                    else:
                        w_sl = p1w.tile([P, CC, P], f32r,
                                        tag=f"{wtag}{p_i % 2}")
                        nc.sync.dma_start(
                            w_sl[:], w_r[:, :, p_i * P:(p_i + 1) * P])
                    for half in range(2):
                        pq = p1q.tile([P, 1024], f32, tag="pq")
                        for s5 in range(2):
                            for cc in range(CC):
                                nc.tensor.matmul(
                                    pq[:, s5 * 512:(s5 + 1) * 512],
                                    w_sl[:, cc, :],
                                    xT_sb[:, cc,
                                          half * 1024 + s5 * 512:
                                          half * 1024 + (s5 + 1) * 512],
                                    start=(cc == 0), stop=(cc == CC - 1))
                        nc.vector.tensor_copy(
                            dst[:, p_i, half * 1024:(half + 1) * 1024],
                            pq[:])

        # v = x @ wv^T in [t, c_local] layout (wv DMA overlaps q/k compute)
        with tc.tile_pool(name=f"p1wv{r}", bufs=1) as p1wv:
            wv_sb = p1wv.tile([P, CC, CL], f32r, tag="wv")
            for cc in range(CC):
                nc.gpsimd.dma_start(wv_sb[:, cc, :], wvT_r[:, cc, :])
            for t_c in range(TC):
                pv = p1ps.tile([P, CL], f32, tag="pv")
                for cc in range(CC):
                    nc.tensor.matmul(
                        pv[:],
                        xT_sb[:, cc, t_c * P:(t_c + 1) * P],
                        wv_sb[:, cc, :],
                        start=(cc == 0), stop=(cc == CC - 1))
                nc.vector.tensor_copy(
                    v_sb[:, t_c, :, 0:HD],
                    pv[:].rearrange("p (h d) -> p h d", h=NH_LOCAL))


def _emit_attention(nc, tc, mybir, r, qT_sb, kT_sb, v_sb, tri_sb, ones_sb, aT_sb):
    """Per (head, i-half) units. aT psum is [65, 1024] (2 banks) double
    buffered, so consecutive units overlap on PE/ACT while the previous
    unit's normalize drains."""
    f32 = mybir.dt.float32
    f32r = mybir.dt.float32r
    EXP = mybir.ActivationFunctionType.Exp
    MULT = mybir.AluOpType.mult
    with tc.tile_pool(name=f"p2{r}", bufs=2) as p2, \
         tc.tile_pool(name=f"p2pt{r}", bufs=4) as p2pt, \
         tc.tile_pool(name=f"p2d{r}", bufs=4, space="DRAM") as p2d, \
         tc.tile_pool(name=f"p2aps{r}", bufs=1, space="PSUM") as p2aps, \
         tc.tile_pool(name=f"p2sps{r}", bufs=2, space="PSUM") as p2sps:
        def _normalize(aT_ps, p_i, par, h0):
            # 1/s row -> broadcast to 64 partitions via K=1 PE matmuls
            # (ones column x reciprocal row), then one multiply into aT_sb.
            rr = p2.tile([P, 1024], f32r, tag="rr")
            with nc.allow_low_precision(reason="f32r recip feeds f32r matmul"):
                nc.vector.reciprocal(rr[64:65, :], aT_ps[64:65, :])
            bc = p2sps.tile([P, 1024], f32, tag="st")
            for s5 in range(0, 1024, 512):
                nc.tensor.matmul(bc[0:HD, s5:s5 + 512], ones_sb[64:65, :],
                                 rr[64:65, s5:s5 + 512], start=True, stop=True)
            rb = p2.tile([HD, 1024], f32, tag="rb")
            nc.vector.tensor_copy(rb[:], bc[0:HD, :])
            if par == 0:
                nc.vector.tensor_tensor(
                    aT_sb[0:HD, p_i, h0:h0 + 1024],
                    aT_ps[0:HD, :], rb[:], MULT)
            else:
                t64 = p2.tile([HD, 1024], f32r, tag="t64")
                nc.vector.tensor_tensor(
                    t64[:], aT_ps[0:HD, :], rb[:], MULT)
                nc.sync.dma_start(
                    aT_sb[HD:P, p_i, h0:h0 + 1024], t64[:])

        for p_i in range(NPAIR):
            for half in range(2):
                h0, h1 = half * 1024, (half + 1) * 1024
                aTs = [p2aps.tile([HD + 1, 1024], f32, tag=f"aT{e}",
                                  name=f"aT{e}_{p_i}_{half}")
                       for e in range(2)]
                jc_end = 8 if half == 0 else 16
                for jc in range(jc_end):
                    w0 = max(h0, 512 * (jc // 4))
                    off = max(0, P * jc - w0)
                    wlen = h1 - w0
                    pts, sts = [], []
                    # scores for both heads back-to-back: alternating PE row
                    # groups (0-1 vs 2-3) let LDWEIGHTS pull ahead
                    for par in range(2):
                        prow = 64 * par
                        st = p2sps.tile([P, 1024], f32, tag="st")
                        for s5 in range(0, wlen, 512):
                            nc.tensor.matmul(
                                st[:, s5:s5 + 512],
                                kT_sb[prow:prow + HD, p_i,
                                      jc * P:(jc + 1) * P],
                                qT_sb[prow:prow + HD, p_i,
                                      w0 + s5:w0 + s5 + 512],
                                start=True, stop=True)
                        sts.append(st)
                    for par in range(2):
                        pt = p2pt.tile([P, 1024], f32r, tag="pt")
                        if off:
                            nc.vector.memset(pt[:, :off].bitcast(f32), 0.0)
                        nc.scalar.activation(
                            pt[:, off:wlen], sts[par][:, off:wlen],
                            EXP, scale=0.125)
                        if off or P * jc == w0:
                            nc.vector.tensor_tensor(
                                pt[:, off:off + P], pt[:, off:off + P],
                                tri_sb[:], MULT)
                        pts.append(pt)
                    for par in range(2):
                        h = 2 * p_i + par
                        for s5 in range(0, wlen, 512):
                            i0 = w0 + s5
                            it = i0 // 512
                            nc.tensor.matmul(
                                aTs[par][:, i0 - h0:i0 - h0 + 512],
                                v_sb[:, jc, h, :],
                                pts[par][:, s5:s5 + 512],
                                start=(jc == 0), stop=(jc == 4 * it + 3))
                for par in range(2):
                    _normalize(aTs[par], p_i, par, h0)


def _emit_out_proj(nc, tc, mybir, r, yT_r, aT_sb, wo_sb):
    f32 = mybir.dt.float32
    with tc.tile_pool(name=f"p3{r}", bufs=4) as p3, \
         tc.tile_pool(name=f"p3ps{r}", bufs=4, space="PSUM") as p3ps:
        for fc in range(CC):
            for tt in range(TT):
                py = p3ps.tile([P, 512], f32, tag="py")
                for cc in range(NPAIR):
                    nc.tensor.matmul(
                        py[:],
                        wo_sb[:, cc, fc * P:(fc + 1) * P],
                        aT_sb[:, cc, tt * 512:(tt + 1) * 512],
                        start=(cc == 0), stop=(cc == NPAIR - 1))
                yst = p3.tile([P, 512], f32, tag="yst")
                nc.vector.tensor_copy(yst[:], py[:])
                eng = nc.sync if (fc * TT + tt) % 2 == 0 else nc.gpsimd
                eng.dma_start(
                    yT_r[:, fc, tt * 512:(tt + 1) * 512], yst[:])


def _build(repeats=1):
    import concourse.bacc as bacc
    import concourse.mybir as mybir
    import concourse.tile as tile
    from contextlib import ExitStack

    f32 = mybir.dt.float32
    f32r = mybir.dt.float32r

    nc = bacc.Bacc("TRN2", target_bir_lowering=False, debug=False)

    xT = nc.dram_tensor("xT", (D, T), f32r, kind="ExternalInput")
    wqT = nc.dram_tensor("wqT", (D, CL), f32r, kind="ExternalInput")
    wkT = nc.dram_tensor("wkT", (D, CL), f32r, kind="ExternalInput")
    wvT = nc.dram_tensor("wvT", (D, CL), f32r, kind="ExternalInput")
    woT = nc.dram_tensor("woT", (CL, D), f32r, kind="ExternalInput")
    tri = nc.dram_tensor("tri", (P, P), f32, kind="ExternalInput")
    yT = nc.dram_tensor("yT", (D, T), f32, kind="ExternalOutput")

    xT_r = xT.ap().rearrange("(o p) t -> p o t", p=P)
    wqT_r = wqT.ap().rearrange("(o p) f -> p o f", p=P)
    wkT_r = wkT.ap().rearrange("(o p) f -> p o f", p=P)
    wvT_r = wvT.ap().rearrange("(o p) f -> p o f", p=P)
    woT_r = woT.ap().rearrange("(o p) f -> p o f", p=P)
    yT_r = yT.ap().rearrange("(o p) t -> p o t", p=P)

    with tile.TileContext(nc) as tc, ExitStack() as outer:
        persist = outer.enter_context(tc.tile_pool(name="persist", bufs=1))
        qT_sb = persist.tile([P, NPAIR, T], f32r, tag="qT")
        kT_sb = persist.tile([P, NPAIR, T], f32r, tag="kT")
        v_sb = persist.tile([P, TC, NH_LOCAL, HD + 1], f32r, tag="v")
        tri_sb = persist.tile([P, P], f32, tag="tri")
        nc.sync.dma_start(tri_sb[:], tri.ap())
        ones_sb = persist.tile([P, HD], f32r, tag="ones")
        nc.vector.memset(ones_sb[:].bitcast(f32), 1.0)

        for r in range(repeats):
            nc.vector.memset(v_sb[:, :, :, HD:HD + 1].bitcast(f32), 1.0)
            _emit_qkv(nc, tc, mybir, r, (xT_r, wqT_r, wkT_r, wvT_r),
                      qT_sb, kT_sb, v_sb)
            with tc.tile_pool(name=f"aT{r}", bufs=1) as aTp, \
                 tc.tile_pool(name=f"wo{r}", bufs=1) as wop:
                aT_sb = aTp.tile([P, NPAIR, T], f32r, tag="aT")
                wo_sb = wop.tile([P, NPAIR, D], f32r, tag="wo")
                nc.sync.dma_start(wo_sb[:], woT_r)
                _emit_attention(nc, tc, mybir, r, qT_sb, kT_sb, v_sb,
                                tri_sb, ones_sb, aT_sb)
                _emit_out_proj(nc, tc, mybir, r, yT_r, aT_sb, wo_sb)

    nc.compile()
    return nc


def kernel(x, w_qkv, w_out):
    from concourse import bass_utils

    if "nc" not in _CACHE:
        _CACHE["nc"] = _build()
    nc = _CACHE["nc"]

    x = np.asarray(x, dtype=np.float32)
    w_qkv = np.asarray(w_qkv, dtype=np.float32)
    w_out = np.asarray(w_out, dtype=np.float32)
    tri = np.triu(np.ones((P, P), dtype=np.float32))

    in_maps = []
    for c in range(8):
        b, g = c // 2, c % 2
        sl = slice(CL * g, CL * g + CL)
        in_maps.append({
            "xT": np.ascontiguousarray(x[b].T),
            "wqT": np.ascontiguousarray(w_qkv[0 * D:1 * D][sl].T),
            "wkT": np.ascontiguousarray(w_qkv[1 * D:2 * D][sl].T),
            "wvT": np.ascontiguousarray(w_qkv[2 * D:3 * D][sl].T),
            "woT": np.ascontiguousarray(w_out[:, sl].T),
            "tri": tri,
        })

    res = bass_utils.run_bass_kernel_spmd(nc, in_maps, core_ids=list(range(8)))
    outs = res.results

    y = np.empty((B, T, D), dtype=np.float32)
    for b in range(B):
        y[b] = (outs[2 * b]["yT"] + outs[2 * b + 1]["yT"]).T
    return y



# revision 19
# speedup vs baseline: 2.1771x; 2.1771x over previous
"""Causal self-attention on 8 axon-tunneled TRN2 NeuronCores.

Sharding: core c -> (batch b = c//2, head-group g = c%2); host sums the two
head-group partial yT outputs per batch. bf16 storage / fp32 PSUM accumulate
(bf16 matmuls stream 1 row/cycle like f32r, but halve DMA, SBUF and DVE
cost). Transpose-free S^T attention layout; softmax denominator via a
ones-column in v. Per (head-pair, 512-i-window) attention units with
causally-trimmed diagonal tiles; two heads packed per score PSUM tile so one
exp covers both; additive -1e9 causal mask applied in PSUM (Pool engine) at
unit START (diagonal tiles are emitted first so mask+exp latency hides under
full-block score streaming; their AVs are deferred to the unit end). Softmax
normalize is lazy and fully off-PE: the unnormalized PSUM tile is drained to
SBUF immediately (releasing the bank for the next unit), then DVE reciprocal
-> gpsimd partition_broadcast -> DVE multiply off the critical path.
Out-proj tiles are software-pipelined into the next attention window's unit
stream; the last window's pair-3 B-half feeds out-proj via a K=64 matmul
from the staging tile (skipping the cross-partition DMA on the tail)."""
import numpy as np

B, T, D = 4, 2048, 1024
NH_LOCAL = 8
HD = 64
CL = 512
P = 128
CC = D // P
NPAIR = 4
NW = 4
W = 512

_CACHE = {}


def _emit_qkv(nc, tc, mybir, r, xT_sb, wq_sb, wk_sb, wv_sb, qT_sb, kT_sb, v_sb):
    f32 = mybir.dt.float32
    COPY = mybir.ActivationFunctionType.Copy
    with tc.tile_pool(name=f"p1ps{r}", bufs=6, space="PSUM") as pps:
        for w in range(NW):
            t0 = w * W
            for w_sb, dst, tag in ((wq_sb, qT_sb, "q"), (wk_sb, kT_sb, "k")):
                for p in range(NPAIR):
                    ps = pps.tile([P, W], f32, tag="pq")
                    for cc in range(CC):
                        nc.tensor.matmul(
                            ps[:], w_sb[:, cc, p * P:(p + 1) * P],
                            xT_sb[:, cc, t0:t0 + W],
                            start=(cc == 0), stop=(cc == CC - 1))
                    if tag == "q":
                        nc.vector.tensor_copy(dst[:, p, t0:t0 + W], ps[:])
                    else:
                        nc.scalar.activation(dst[:, p, t0:t0 + W], ps[:], COPY)
            for tcb in range(4):
                t_c = w * 4 + tcb
                ps = pps.tile([P, W], f32, tag="pq")
                for cc in range(CC):
                    nc.tensor.matmul(
                        ps[:], xT_sb[:, cc, t_c * P:(t_c + 1) * P],
                        wv_sb[:, cc, :],
                        start=(cc == 0), stop=(cc == CC - 1))
                nc.vector.tensor_copy(
                    v_sb[:, t_c, :, 0:HD],
                    ps[:].rearrange("p (h d) -> p h d", h=NH_LOCAL))


def _emit_attention(nc, tc, mybir, r, qT_sb, kT_sb, v_sb, ml_sb, mr_sb,
                    aT_sb, wo_sb, woB_sb, yT_r):
    f32 = mybir.dt.float32
    bf16 = mybir.dt.bfloat16
    EXP = mybir.ActivationFunctionType.Exp
    MULT = mybir.AluOpType.mult
    with tc.tile_pool(name=f"p2st{r}", bufs=2, space="PSUM") as pst, \
         tc.tile_pool(name=f"p2aT{r}", bufs=1, space="PSUM") as paT, \
         tc.tile_pool(name=f"p2py{r}", bufs=2, space="PSUM") as ppy, \
         tc.tile_pool(name=f"p2pt{r}", bufs=4) as ppt, \
         tc.tile_pool(name=f"p2n{r}", bufs=2) as pn, \
         tc.tile_pool(name=f"p2y{r}", bufs=4) as pys:
        op_queue = []
        t64_last = [None]

        def emit_op(n):
            for _ in range(n):
                if not op_queue:
                    return
                fc, t0, last_w = op_queue.pop(0)
                py = ppy.tile([P, W], f32, tag="py")
                for pair in range(NPAIR - 1 if last_w else NPAIR):
                    nc.tensor.matmul(
                        py[:], wo_sb[:, pair, fc * P:(fc + 1) * P],
                        aT_sb[:, pair, t0:t0 + W],
                        start=(pair == 0), stop=False)
                if last_w:
                    # pair 3: A half from aT_sb, B half straight from the
                    # staging tile (skips the cross-partition DMA)
                    nc.tensor.matmul(
                        py[:], wo_sb[0:HD, 3, fc * P:(fc + 1) * P],
                        aT_sb[0:HD, 3, t0:t0 + W], start=False, stop=False)
                    nc.tensor.matmul(
                        py[:], woB_sb[:, 3, fc * P:(fc + 1) * P],
                        t64_last[0][:], start=False, stop=True)
                else:
                    nc.tensor.matmul(
                        py[:], wo_sb[:, NPAIR - 1, fc * P:(fc + 1) * P],
                        aT_sb[:, NPAIR - 1, t0:t0 + W],
                        start=False, stop=True)
                yst = pys.tile([P, W], bf16, tag="yst")
                nc.vector.tensor_copy(yst[:], py[:])
                nc.sync.dma_start(yT_r[:, fc, t0:t0 + W], yst[:])

        for w in range(NW):
            t0 = w * W
            nfull = 4 * w
            for p in range(NPAIR):
                aTA = paT.tile([HD + 1, W], f32, tag="aTA",
                               name=f"aTA_{p}_{w}")
                aTB = paT.tile([HD + 1, W], f32, tag="aTB",
                               name=f"aTB_{p}_{w}")

                def av(ptj, jc, c0, wd):
                    for par, aT in ((0, aTA), (1, aTB)):
                        nc.tensor.matmul(
                            aT[:, c0:W], v_sb[:, jc, 2 * p + par, :],
                            ptj[:, par * wd:(par + 1) * wd],
                            start=(jc == 0), stop=(jc == nfull + 3))

                # full blocks then diagonal tiles, one-deep AV pipeline:
                # AV(k) is emitted after exp(k+1) so the PE streams
                # scores(k+1) while ACT computes exp(k). The causal mask is
                # a rank-127 matmul (-1e9*max(0, j-i) = ml^T @ mr)
                # accumulated into the diagonal PSUM block — no cross-engine
                # hop before the exp.
                pending = None
                for jc in range(nfull):
                    stt = pst.tile([P, 2 * W], f32, tag="st",
                                   name=f"stf{jc}_{p}_{w}")
                    for par, prow in ((0, 0), (1, HD)):
                        nc.tensor.matmul(
                            stt[:, par * W:(par + 1) * W],
                            kT_sb[prow:prow + HD, p, jc * P:(jc + 1) * P],
                            qT_sb[prow:prow + HD, p, t0:t0 + W],
                            start=True, stop=True)
                    pt = ppt.tile([P, 2 * W], bf16, tag="pt")
                    nc.scalar.activation(pt[:], stt[:], EXP, scale=0.125)
                    if pending is not None:
                        av(*pending)
                    pending = (pt, jc, 0, W)
                for d in range(4):
                    jc = nfull + d
                    wd = W - d * P
                    stt = pst.tile([P, 2 * W], f32, tag="st",
                                   name=f"std{d}_{p}_{w}")
                    for par, prow in ((0, 0), (1, HD)):
                        nc.tensor.matmul(
                            stt[:, par * wd:(par + 1) * wd],
                            kT_sb[prow:prow + HD, p, jc * P:(jc + 1) * P],
                            qT_sb[prow:prow + HD, p, t0 + d * P:t0 + W],
                            start=True, stop=False)
                        nc.tensor.matmul(
                            stt[:, par * wd:par * wd + P],
                            ml_sb[:], mr_sb[:], start=False, stop=True)
                    pt = ppt.tile([P, 2 * W], bf16, tag="pt")
                    nc.scalar.activation(pt[:, 0:2 * wd], stt[:, 0:2 * wd],
                                         EXP, scale=0.125)
                    if pending is not None:
                        av(*pending)
                    pending = (pt, jc, d * P, wd)
                av(*pending)

                # lazy normalize: drain PSUM to SBUF right away (frees the
                # bank for the next unit), then 1/s entirely off-PE.
                aTuA = pn.tile([HD + 1, W], bf16, tag="aTuA")
                nc.vector.tensor_copy(aTuA[:], aTA[:])
                aTuB = pn.tile([HD + 1, W], bf16, tag="aTuB")
                nc.vector.tensor_copy(aTuB[:], aTB[:])
                rrA = pn.tile([P, W], bf16, tag="rrA")
                rrB = pn.tile([P, W], bf16, tag="rrB")
                with nc.allow_low_precision(reason="bf16 softmax denom"):
                    nc.vector.reciprocal(rrA[HD:HD + 1, :],
                                         aTuA[HD:HD + 1, :])
                    nc.vector.reciprocal(rrB[HD:HD + 1, :],
                                         aTuB[HD:HD + 1, :])
                rbA = pn.tile([HD, W], bf16, tag="rbA")
                nc.gpsimd.partition_broadcast(rbA[:], rrA[HD:HD + 1, :])
                rbB = pn.tile([HD, W], bf16, tag="rbB")
                nc.gpsimd.partition_broadcast(rbB[:], rrB[HD:HD + 1, :])
                nc.vector.tensor_tensor(
                    aT_sb[0:HD, p, t0:t0 + W], aTuA[0:HD, :], rbA[:], MULT)
                t64 = pn.tile([HD, W], bf16, tag="t64")
                nc.vector.tensor_tensor(t64[:], aTuB[0:HD, :], rbB[:], MULT)
                if w == NW - 1 and p == NPAIR - 1:
                    t64_last[0] = t64
                else:
                    nc.sync.dma_start(aT_sb[HD:P, p, t0:t0 + W], t64[:])

                if w > 0:
                    emit_op(2)
            op_queue += [(fc, t0, w == NW - 1) for fc in range(CC)]
        emit_op(len(op_queue))


def _build(repeats=1):
    import concourse.bacc as bacc
    import concourse.mybir as mybir
    import concourse.tile as tile
    from contextlib import ExitStack

    f32 = mybir.dt.float32
    bf16 = mybir.dt.bfloat16

    nc = bacc.Bacc("TRN2", target_bir_lowering=False, debug=False)

    xT = nc.dram_tensor("xT", (D, T), bf16, kind="ExternalInput")
    wqT = nc.dram_tensor("wqT", (D, CL), bf16, kind="ExternalInput")
    wkT = nc.dram_tensor("wkT", (D, CL), bf16, kind="ExternalInput")
    wvT = nc.dram_tensor("wvT", (D, CL), bf16, kind="ExternalInput")
    woT = nc.dram_tensor("woT", (CL, D), bf16, kind="ExternalInput")
    ml = nc.dram_tensor("ml", (P, P), bf16, kind="ExternalInput")
    mr = nc.dram_tensor("mr", (P, P), bf16, kind="ExternalInput")
    yT = nc.dram_tensor("yT", (D, T), bf16, kind="ExternalOutput")

    xT_r = xT.ap().rearrange("(o p) t -> p o t", p=P)
    wqT_r = wqT.ap().rearrange("(o p) f -> p o f", p=P)
    wkT_r = wkT.ap().rearrange("(o p) f -> p o f", p=P)
    wvT_r = wvT.ap().rearrange("(o p) f -> p o f", p=P)
    woT_r = woT.ap().rearrange("(o p) f -> p o f", p=P)
    yT_r = yT.ap().rearrange("(o p) t -> p o t", p=P)

    with tile.TileContext(nc) as tc, ExitStack() as outer:
        persist = outer.enter_context(tc.tile_pool(name="persist", bufs=1))
        xT_sb = persist.tile([P, CC, T], bf16, tag="xT")
        qT_sb = persist.tile([P, NPAIR, T], bf16, tag="qT")
        kT_sb = persist.tile([P, NPAIR, T], bf16, tag="kT")
        aT_sb = persist.tile([P, NPAIR, T], bf16, tag="aT")
        v_sb = persist.tile([P, 16, NH_LOCAL, HD + 1], bf16, tag="v")
        wq_sb = persist.tile([P, CC, CL], bf16, tag="wq")
        wk_sb = persist.tile([P, CC, CL], bf16, tag="wk")
        wv_sb = persist.tile([P, CC, CL], bf16, tag="wv")
        wo_sb = persist.tile([P, NPAIR, D], bf16, tag="wo")
        woB_sb = persist.tile([HD, NPAIR, D], bf16, tag="woB")
        ml_sb = persist.tile([P, P], bf16, tag="ml")
        mr_sb = persist.tile([P, P], bf16, tag="mr")

        for r in range(repeats):
            nc.sync.dma_start(wq_sb[:], wqT_r)
            nc.sync.dma_start(wk_sb[:], wkT_r)
            for w in range(NW):
                nc.gpsimd.dma_start(xT_sb[:, :, w * W:(w + 1) * W],
                                    xT_r[:, :, w * W:(w + 1) * W])
            nc.scalar.dma_start(wv_sb[:], wvT_r)
            nc.scalar.dma_start(ml_sb[:], ml.ap())
            nc.scalar.dma_start(mr_sb[:], mr.ap())
            nc.scalar.dma_start(wo_sb[:], woT_r)
            nc.scalar.dma_start(woB_sb[:], woT_r[HD:P, :, :])
            nc.vector.memset(v_sb[:, :, :, HD:HD + 1], 1.0)
            _emit_qkv(nc, tc, mybir, r, xT_sb, wq_sb, wk_sb, wv_sb,
                      qT_sb, kT_sb, v_sb)
            _emit_attention(nc, tc, mybir, r, qT_sb, kT_sb, v_sb, ml_sb,
                            mr_sb, aT_sb, wo_sb, woB_sb, yT_r)

    nc.compile()
    return nc


def _host_inputs(x, w_qkv, w_out, core):
    import ml_dtypes

    bf = ml_dtypes.bfloat16
    b, g = core // 2, core % 2
    sl = slice(CL * g, CL * g + CL)
    k_i = np.arange(P)
    # -1e9*max(0, j-i) == ml^T @ mr with ml[k,j] = -1e9*[k<j], mr[k,i] = [i<=k]
    ml = np.where(k_i[:, None] < k_i[None, :], -1e9, 0.0)
    mr = np.where(k_i[None, :] <= k_i[:, None], 1.0, 0.0)
    return {
        "xT": np.ascontiguousarray(x[b].T).astype(bf),
        "wqT": np.ascontiguousarray(w_qkv[0 * D:1 * D][sl].T).astype(bf),
        "wkT": np.ascontiguousarray(w_qkv[1 * D:2 * D][sl].T).astype(bf),
        "wvT": np.ascontiguousarray(w_qkv[2 * D:3 * D][sl].T).astype(bf),
        "woT": np.ascontiguousarray(w_out[:, sl].T).astype(bf),
        "ml": ml.astype(bf),
        "mr": mr.astype(bf),
    }


def kernel(x, w_qkv, w_out):
    from concourse import bass_utils

    if "nc" not in _CACHE:
        _CACHE["nc"] = _build()
    nc = _CACHE["nc"]

    x = np.asarray(x, dtype=np.float32)
    w_qkv = np.asarray(w_qkv, dtype=np.float32)
    w_out = np.asarray(w_out, dtype=np.float32)

    in_maps = [_host_inputs(x, w_qkv, w_out, c) for c in range(8)]
    res = bass_utils.run_bass_kernel_spmd(nc, in_maps, core_ids=list(range(8)))
    outs = res.results

    y = np.empty((B, T, D), dtype=np.float32)
    for b in range(B):
        y[b] = (outs[2 * b]["yT"].astype(np.float32)
                + outs[2 * b + 1]["yT"].astype(np.float32)).T
    return y
